# revision 1
# baseline (speedup 1.0000x reference)
"""Trainium2 Bass kernel for a dense transformer block (LN -> 16-head causal
attention -> proj+residual -> LN -> FFN+residual), B=8 data-parallel over 8
NeuronCores (one batch element per core).

Matmuls run in fp16 (10 explicit mantissa bits -- near float32r precision
at half the DMA/SBUF cost, full PE speed at any free dim).  The residual
stream, layer norms and softmax bookkeeping stay fp32.  LayerNorm gamma/beta are folded
into the adjacent projection weights on the host (exact algebra), and the
1/sqrt(E) attention scale is folded into wq.

Activation dataflow is transposed ([feature, token]) for the matmul chain;
scores are computed transposed ([key, query]) so softmax denominators come
out of the PV matmul via an appended ones-column in V, removing any need to
transpose the attention probabilities.

SBUF is managed with three rotating big pools (pool size is
sum-over-tags of bufs*slot, reserved from pool alloc to release, stack
per side):
  big2 (2 slots, 32.5KB ea): x_sb -> h_sb -> V_pad -> attnT -> x_re -> h2_sb
  big1 (3 slots, 32KB ea):   hT, QT, KT           (dies after attention)
  bigA (2 slots, right side): x2, h2T             (proj -> end)
"""

import os
import sys

sys.path.insert(0, "/opt/trn_rl_repo")

# a cpu-pinned jax would hide the NeuronCores from the PJRT execution path
if os.environ.get("JAX_PLATFORMS") == "cpu":
    os.environ.pop("JAX_PLATFORMS")

import numpy as np

import concourse.bacc as bacc
import concourse.mybir as mybir
import concourse.tile as tile
from concourse.bass_utils import run_bass_kernel_spmd
from concourse.masks import make_identity

F32 = mybir.dt.float32
F16 = mybir.dt.float16
F8 = mybir.dt.float8e4
DR = mybir.MatmulPerfMode.DoubleRow
AF = mybir.ActivationFunctionType
OP = mybir.AluOpType
AX = mybir.AxisListType

NP_F8 = mybir.dt.np(F8)  # ml_dtypes.float8_e4m3 (IEEE, max 240, min normal 2^-6)

# fp8 stage selection: Q/K + FFN1 (+FFN2) run fp8 DoubleRow at 2x PE rate.
# V and proj stay fp16: their quantization error flows straight into the
# residual stream through a 1024-deep contraction (V bypasses softmax
# averaging for early tokens), measured ~8e-3 rel each vs ~8e-4 for Q/K.
FFN2_FP8 = True

# fp8 weight pre-scales (power of 2, exact): w8 = w * S, descale 1/S applied at
# the PSUM copy-out.  Set by prepare_in_maps before build_nc is called.
_SCALES: dict = {}


def _pow2_scale(arr, target=8.0):
    m = float(np.abs(arr).max())
    if m == 0.0:
        return 1.0
    return 2.0 ** np.floor(np.log2(target / m))

P = 128
E = 1024
H = 16
D = 64
T = 1024
B = 8
F = 4 * E
EC = E // P     # 8 chunks of features
FC = F // P     # 32 chunks of ffn hidden
TBn = T // P    # 8 token blocks of 128
LN_EPS = 1e-5
NEG = -1.0e9
SKIP_NORM = False
STOP_AFTER_ATTN = False
STOP_BEFORE_FFN = False
SIMPLE_ATTN = False


def _emit_ln_stats(nc, pool, src_blk, tag, name, eps_sb):
    """Per-token-block LN stats: src_blk [128, E] -> (rstd, nmr) [128, 1]."""
    sums = pool.tile([P, 1], F32, tag=f"{tag}_sums", name=f"{name}_sums")
    sumsq = pool.tile([P, 1], F32, tag=f"{tag}_sumsq", name=f"{name}_sumsq")
    dump = pool.tile([P, E], F32, tag=f"{tag}_dump", name=f"{name}_dump")
    m2 = pool.tile([P, 1], F32, tag=f"{tag}_m2", name=f"{name}_m2")
    mu = pool.tile([P, 1], F32, tag=f"{tag}_mu", name=f"{name}_mu")
    var = pool.tile([P, 1], F32, tag=f"{tag}_var", name=f"{name}_var")
    rstd = pool.tile([P, 1], F32, tag=f"{tag}_rstd", name=f"{name}_rstd")
    nmr = pool.tile([P, 1], F32, tag=f"{tag}_nmr", name=f"{name}_nmr")
    nc.vector.reduce_sum(sums[:], src_blk, axis=AX.X)
    nc.scalar.activation(dump[:], src_blk, AF.Square, accum_out=sumsq[:])
    nc.vector.tensor_scalar_mul(mu[:], sums[:], 1.0 / E)
    nc.vector.tensor_tensor(m2[:], mu[:], mu[:], op=OP.mult)
    nc.vector.scalar_tensor_tensor(
        var[:], sumsq[:], 1.0 / E, m2[:], op0=OP.mult, op1=OP.subtract
    )
    nc.scalar.activation(rstd[:], var[:], AF.Sqrt, bias=eps_sb[:])
    nc.vector.reciprocal(rstd[:], rstd[:])
    nc.vector.scalar_tensor_tensor(
        nmr[:], mu[:], -1.0, rstd[:], op0=OP.mult, op1=OP.mult
    )
    return rstd, nmr


def _emit_tp_block(nc, ps_tp, tb, src, dst, ident, name, dst2=None):
    """One token block of src [128, TBn, E] (token-major) -> dst[, dst2]
    [128, EC, T] (feature-major) via PE transposes of 128x128 blocks; the
    copies out of PSUM alternate engines (and cast to the dst dtype)."""
    for j in range(EC):
        tp = ps_tp.tile([P, P], src.dtype, tag="tp", name=f"{name}_tp_{j}_{tb}")
        nc.tensor.transpose(tp[:], src[:, tb, j * P : (j + 1) * P], ident[:])
        if (j + tb) % 2 == 0:
            nc.vector.tensor_copy(dst[:, j, tb * P : (tb + 1) * P], tp[:])
            if dst2 is not None:
                nc.scalar.copy(dst2[:, j, tb * P : (tb + 1) * P], tp[:])
        else:
            nc.scalar.copy(dst[:, j, tb * P : (tb + 1) * P], tp[:])
            if dst2 is not None:
                nc.vector.tensor_copy(dst2[:, j, tb * P : (tb + 1) * P], tp[:])


def build_nc(reps=1):
    nc = bacc.Bacc(None, target_bir_lowering=False)

    assert _SCALES, "prepare_in_maps must run before build_nc (sets fp8 scales)"
    ds_q = 1.0 / _SCALES["wqt"]
    ds_k = 1.0 / _SCALES["wkt"]
    ds_1 = 1.0 / _SCALES["w1t"]
    ds_2 = 1.0 / _SCALES["w2t"]
    F8_2 = F8 if FFN2_FP8 else F16

    x_d = nc.dram_tensor("x", [T, E], F32, kind="ExternalInput")
    # weights pre-tiled on host to [out_chunk, p(=in%128), in_chunk, col]
    wqt_d = nc.dram_tensor("wqt", [EC, P, EC, P], F8, kind="ExternalInput")
    wkt_d = nc.dram_tensor("wkt", [EC, P, EC, P], F8, kind="ExternalInput")
    wvt_d = nc.dram_tensor("wvt", [EC, P, EC, P], F16, kind="ExternalInput")
    wpt_d = nc.dram_tensor("wpt", [EC, P, EC, P], F16, kind="ExternalInput")
    w1t_d = nc.dram_tensor("w1t", [FC, P, EC, P], F8, kind="ExternalInput")
    w2t_d = nc.dram_tensor("w2t", [EC, P, FC, P], F8_2, kind="ExternalInput")
    bq_d = nc.dram_tensor("bq", [P, EC], F32, kind="ExternalInput")
    bk_d = nc.dram_tensor("bk", [P, EC], F32, kind="ExternalInput")
    bp_d = nc.dram_tensor("bp", [P, EC], F32, kind="ExternalInput")
    b1_d = nc.dram_tensor("b1", [P, FC], F32, kind="ExternalInput")
    b2_d = nc.dram_tensor("b2", [P, EC], F32, kind="ExternalInput")
    out_d = nc.dram_tensor("out", [T, E], F32, kind="ExternalOutput")

    with tile.TileContext(nc) as tc:
      for _rep in range(reps):
        ps_big = tc.alloc_tile_pool(name="ps_big", bufs=2, space="PSUM")
        ps_attn = tc.alloc_tile_pool(name="ps_attn", bufs=2, space="PSUM")
        ps_tp = tc.alloc_tile_pool(name="ps_tp", bufs=2, space="PSUM")
        constp = tc.alloc_tile_pool(name="const", bufs=1)

        ident = constp.tile([P, P], F32, name="ident")
        make_identity(nc, ident[:])
        ident16 = constp.tile([P, P], F16, name="ident16")
        make_identity(nc, ident16[:])
        # mask[s, q] = 0 if s <= q else NEG  (within a diagonal 128 block)
        mask = constp.tile([P, P], F32, name="mask")
        nc.gpsimd.memset(mask[:], 0.0)
        nc.gpsimd.affine_select(
            out=mask[:],
            in_=mask[:],
            compare_op=OP.is_ge,
            fill=NEG,
            base=0,
            pattern=[[1, P]],
            channel_multiplier=-1,
        )
        eps_sb = constp.tile([P, 1], F32, name="eps_sb")
        nc.vector.memset(eps_sb[:], LN_EPS)
        bq_sb = constp.tile([P, EC], F32, name="bq_sb")
        bk_sb = constp.tile([P, EC], F32, name="bk_sb")
        bp_sb = constp.tile([P, EC], F32, name="bp_sb")
        b1_sb = constp.tile([P, FC], F32, name="b1_sb")
        b2_sb = constp.tile([P, EC], F32, name="b2_sb")
        nc.sync.dma_start(bq_sb[:], bq_d[:])
        nc.sync.dma_start(bk_sb[:], bk_d[:])
        nc.sync.dma_start(bp_sb[:], bp_d[:])
        nc.sync.dma_start(b1_sb[:], b1_d[:])
        nc.sync.dma_start(b2_sb[:], b2_d[:])

        big2 = tc.alloc_tile_pool(name="big2", bufs=3)
        big1 = tc.alloc_tile_pool(name="big1", bufs=1)

        # ---- Phase 1: LN1 + transpose, software-pipelined per token block:
        # stats(tb) fills engine bubbles left by normalize+transpose(tb-1) ----
        ln1p = tc.alloc_tile_pool(name="ln1", bufs=2)
        x_sb = big2.tile([P, TBn, E], F32, tag="b2", name="x_sb")
        for tb in range(TBn):
            nc.sync.dma_start(x_sb[:, tb], x_d[tb * P : (tb + 1) * P, :])
        h_sb = big2.tile([P, TBn, E], F16, tag="b2", name="h_sb")
        hT = big1.tile([P, EC, T], F16, tag="b1", name="hT")
        hT8 = big1.tile([P, EC, T], F8, tag="b1_8", name="hT8")

        ln1_stats = {}
        for tb in range(TBn + 1):
            if tb < TBn:
                ln1_stats[tb] = _emit_ln_stats(
                    nc, ln1p, x_sb[:, tb], "ln1", f"ln1_{tb}", eps_sb
                )
            if tb >= 1:
                rstd, nmr = ln1_stats[tb - 1]
                nc.scalar.activation(
                    h_sb[:, tb - 1], x_sb[:, tb - 1], AF.Identity,
                    bias=nmr[:], scale=rstd[:],
                )
                _emit_tp_block(nc, ps_tp, tb - 1, h_sb, hT, ident16, "h")
                # fp8 shadow of hT for the Q/K DoubleRow matmuls, produced on
                # the otherwise-idle gpsimd engine (SBUF->SBUF)
                nc.gpsimd.tensor_copy(
                    hT8[:, :, (tb - 1) * P : tb * P],
                    hT[:, :, (tb - 1) * P : tb * P],
                )
        ln1p.release()

        # release LN transpose psum before the merged phase (PSUM budget)
        ps_tp.release()

        # ---- Phase 2+3: QKV + attention, interleaved per head-quad ----
        # For each quad of 4 heads (2 feature chunks): compute QT/KT/V, then
        # run attention while draining next quad's QKV matmuls between
        # attention steps so the in-order PE never stalls on ACT's exps.
        ps_sc = tc.alloc_tile_pool(name="ps_sc", bufs=2, space="PSUM")
        wqkp = tc.alloc_tile_pool(name="wqk", bufs=3)
        qkp = tc.alloc_tile_pool(name="qkp", bufs=8)
        expp = tc.alloc_tile_pool(name="expp", bufs=6)
        rp = tc.alloc_tile_pool(name="rp", bufs=2)

        V_pad = big2.tile([P, TBn, H * (D + 1)], F16, tag="b2", name="V_pad")
        attnT = big2.tile([P, EC, T], F16, tag="b2", name="attnT")
        vpr = V_pad[:].rearrange("p tb (h dd) -> p tb h dd", dd=D + 1)
        ones16 = constp.tile([P, TBn, H], F32, name="ones16")
        nc.vector.memset(ones16[:], 1.0)
        nc.vector.tensor_copy(vpr[:, :, :, D : D + 1], ones16[:, :, :, None])

        def emit_qkv_quad(q):
            """DMA the quad's weights, allocate QT/KT tiles, and return
            (qt, kt, thunks); each thunk emits one psum accumulation group."""
            wq_t = wqkp.tile([P, 2, EC, P], F8, tag="wqk", name=f"wq_{_rep}_{q}")
            nc.sync.dma_start(
                wq_t[:],
                wqt_d[2 * q : 2 * q + 2].rearrange("jj p i c -> p jj i c"),
            )
            wk_t = wqkp.tile([P, 2, EC, P], F8, tag="wqk", name=f"wk_{_rep}_{q}")
            nc.sync.dma_start(
                wk_t[:],
                wkt_d[2 * q : 2 * q + 2].rearrange("jj p i c -> p jj i c"),
            )
            wv_t = wqkp.tile([P, 2, EC, P], F16, tag="wv", name=f"wv_{_rep}_{q}")
            nc.sync.dma_start(
                wv_t[:],
                wvt_d[2 * q : 2 * q + 2].rearrange("jj p i c -> p jj i c"),
            )
            qt, kt, thunks = {}, {}, []
            for jj in range(2):
                j = 2 * q + jj
                qt[jj] = qkp.tile([P, T], F16, tag="qk", name=f"QT_{_rep}_{j}")
                kt[jj] = qkp.tile([P, T], F16, tag="qk", name=f"KT_{_rep}_{j}")
                for wsel, (w_t, dstt, bias_sb, dsc) in enumerate(
                    [(wq_t, qt[jj], bq_sb, ds_q), (wk_t, kt[jj], bk_sb, ds_k)]
                ):
                    for tq in range(2):
                        def _qk(w_t=w_t, dstt=dstt, bias_sb=bias_sb, jj=jj,
                                j=j, tq=tq, wsel=wsel, dsc=dsc):
                            psm = ps_big.tile(
                                [P, 512], F32, tag="mm",
                                name=f"qk_ps_{_rep}_{wsel}_{j}_{tq}",
                            )
                            for i in range(0, EC, 2):
                                nc.tensor.matmul(
                                    psm[:],
                                    w_t[:, jj, i : i + 2, :],
                                    hT8[:, i : i + 2, tq * 512 : (tq + 1) * 512],
                                    start=(i == 0),
                                    stop=(i == EC - 2),
                                    perf_mode=DR,
                                )
                            nc.vector.tensor_scalar(
                                dstt[:, tq * 512 : (tq + 1) * 512],
                                psm[:],
                                dsc,
                                bias_sb[:, j : j + 1],
                                op0=OP.mult,
                                op1=OP.add,
                            )
                        thunks.append(_qk)
            for m in range(TBn):
                def _v(m=m, wv_t=wv_t, q=q):
                    psm = ps_big.tile(
                        [P, 512], F32, tag="mm", name=f"v_ps_{_rep}_{q}_{m}"
                    )
                    for i in range(EC):
                        nc.tensor.matmul(
                            psm[:, :256],
                            hT[:, i, m * P : (m + 1) * P],
                            wv_t[:, :, i, :],
                            start=(i == 0),
                            stop=(i == EC - 1),
                        )
                    dst4 = vpr[:, m, q * 4 : (q + 1) * 4, 0:D]
                    src4 = psm[:, :256].rearrange("p (h d) -> p h d", d=D)
                    nc.vector.tensor_copy(dst4, src4)
                thunks.append(_v)
            return qt, kt, thunks

        qt_cur, kt_cur, thunks0 = emit_qkv_quad(0)
        for t in thunks0:
            t()
        for q in range(4):
            if q < 3:
                qt_next, kt_next, bg = emit_qkv_quad(q + 1)
            else:
                qt_next = kt_next = None
                bg = []
            for t in bg:
                t()
            bg = []
            bgi = 0
            for jj in range(2):
                j = 2 * q + jj
                QTj, KTj = qt_cur[jj], kt_cur[jj]
                for qc in range(2):
                    nblk = 4 * qc + 4
                    aps = [
                        ps_attn.tile(
                            [D + 1, 512], F32, tag="attn",
                            name=f"at_ps_{_rep}_{j}_{qc}_{hh}",
                        )
                        for hh in range(2)
                    ]
                    exps = {}
                    offs = {}
                    for sb in range(nblk):
                        off = max(0, (sb - 4 * qc) * P)
                        w = 512 - off
                        offs[sb] = off
                        # both heads share one 2-bank psum tile: h0 cols
                        # [off,512), h1 cols [512+off,1024) -> one fused mask
                        # add and one fused exp per step instead of two
                        sc = ps_sc.tile(
                            [P, 1024], F32, tag="sc",
                            name=f"sc_{_rep}_{j}_{qc}_{sb}",
                        )
                        for hh in range(2):
                            nc.tensor.matmul(
                                sc[:, hh * 512 + off : (hh + 1) * 512],
                                KTj[hh * D : (hh + 1) * D, sb * P : (sb + 1) * P],
                                QTj[hh * D : (hh + 1) * D,
                                    qc * 512 + off : (qc + 1) * 512],
                                start=True,
                                stop=True,
                                tile_position=None,
                            )
                        scv = sc[:].rearrange("p (g c) -> p g c", g=2)
                        if sb >= 4 * qc:
                            nc.vector.tensor_tensor(
                                scv[:, :, off : off + P],
                                scv[:, :, off : off + P],
                                mask[:, None, :].to_broadcast([P, 2, P]),
                                op=OP.add,
                            )
                        ex = expp.tile(
                            [P, 1024], F16, tag="exp",
                            name=f"ex_{_rep}_{j}_{qc}_{sb}",
                        )
                        exv = ex[:].rearrange("p (g c) -> p g c", g=2)
                        nc.scalar.activation(
                            exv[:, :, off:512], scv[:, :, off:512], AF.Exp
                        )
                        exps[sb] = ex
                        # software-pipeline: PV of sb-1 issued after scores of sb
                        if sb > 0:
                            for hh in range(2):
                                po = offs[sb - 1]
                                nc.tensor.matmul(
                                    aps[hh][:, po:512],
                                    V_pad[:, sb - 1,
                                          (2 * j + hh) * (D + 1)
                                          : (2 * j + hh + 1) * (D + 1)],
                                    exps[sb - 1][:, hh * 512 + po
                                                 : (hh + 1) * 512],
                                    start=(sb - 1 == 0),
                                    stop=False,
                                )
                        # drain one background QKV group for the next quad
                        if bgi < len(bg):
                            bg[bgi]()
                            bgi += 1
                    for hh in range(2):
                        po = offs[nblk - 1]
                        nc.tensor.matmul(
                            aps[hh][:, po:512],
                            V_pad[:, nblk - 1,
                                  (2 * j + hh) * (D + 1)
                                  : (2 * j + hh + 1) * (D + 1)],
                            exps[nblk - 1][:, hh * 512 + po : (hh + 1) * 512],
                            start=(nblk == 1),
                            stop=True,
                        )
                    for hh in range(2):
                        if SKIP_NORM:
                            nc.vector.tensor_copy(
                                attnT[hh * D : (hh + 1) * D, j,
                                      qc * 512 : (qc + 1) * 512],
                                aps[hh][0:D, :],
                            )
                            continue
                        rinv = rp.tile(
                            [1, 512], F32, tag="rinv", name=f"ri_{_rep}_{j}_{qc}_{hh}"
                        )
                        nc.vector.reciprocal(rinv[:], aps[hh][D : D + 1, :])
                        rb = rp.tile(
                            [D, 512], F32, tag="rb", name=f"rb_{_rep}_{j}_{qc}_{hh}"
                        )
                        nc.gpsimd.partition_broadcast(rb[:], rinv[:])
                        nc.vector.tensor_tensor(
                            attnT[hh * D : (hh + 1) * D, j,
                                  qc * 512 : (qc + 1) * 512],
                            aps[hh][0:D, :],
                            rb[:],
                            op=OP.mult,
                        )
            while bgi < len(bg):
                bg[bgi]()
                bgi += 1
            qt_cur, kt_cur = qt_next, kt_next
        rp.release()
        expp.release()
        qkp.release()
        wqkp.release()
        ps_sc.release()
        big1.release()
        ps_tp2 = tc.alloc_tile_pool(name="ps_tp2", bufs=2, space="PSUM")

        if STOP_AFTER_ATTN:
            for c in range(EC):
                nc.sync.dma_start(
                    out_d[c * P : (c + 1) * P, 0:512],
                    attnT[:, c, :].bitcast(F32),
                )
            big2.release()
            constp.release()
            ps_tp2.release()
            ps_attn.release()
            ps_big.release()
            continue

        # ---- Phase 4: proj + residual ----
        pA_x2 = tc.alloc_tile_pool(name="pA_x2", bufs=1, side="right")
        wpp = tc.alloc_tile_pool(name="wpp", bufs=2)
        satp = tc.alloc_tile_pool(name="satp", bufs=2)
        x2 = pA_x2.tile([P, TBn, E], F32, tag="bA_x2", name="x2")
        for c in range(EC):
            if c % 4 == 0:
                wcol = wpp.tile([P, 4, EC, P], F16, tag="wp", name=f"wp_{c // 4}")
                nc.sync.dma_start(
                    wcol[:],
                    wpt_d[c : c + 4].rearrange("jj p i cc -> p jj i cc"),
                )
            saT = satp.tile([P, T], F32, tag="saT", name=f"saT_{c}")
            # shared stationary weights: both token halves per ldweights
            psms = [
                ps_big.tile([P, 512], F32, tag="mm", name=f"pj_ps_{c}_{tq}")
                for tq in range(2)
            ]
            for i in range(EC):
                for tq in range(2):
                    nc.tensor.matmul(
                        psms[tq][:],
                        wcol[:, c % 4, i],
                        attnT[:, i, tq * 512 : (tq + 1) * 512],
                        start=(i == 0),
                        stop=(i == EC - 1),
                    )
            for tq in range(2):
                nc.scalar.activation(
                    saT[:, tq * 512 : (tq + 1) * 512],
                    psms[tq][:],
                    AF.Identity,
                    bias=bp_sb[:, c : c + 1],
                )
            for tb in range(TBn):
                tp = ps_tp2.tile([P, P], F32, tag="tp", name=f"pj_tp_{c}_{tb}")
                nc.tensor.transpose(tp[:], saT[:, tb * P : (tb + 1) * P], ident[:])
                nc.vector.tensor_tensor(
                    x2[:, tb, c * P : (c + 1) * P],
                    x_sb[:, tb, c * P : (c + 1) * P],
                    tp[:],
                    op=OP.add,
                )
        satp.release()
        wpp.release()

        # ---- Phase 5: LN2 + transpose, pipelined per token block ----
        pA_h2 = tc.alloc_tile_pool(name="pA_h2", bufs=1, side="right")
        ln2p = tc.alloc_tile_pool(name="ln2", bufs=2)
        h2T = pA_h2.tile([P, EC, T], F8, tag="bA_h2T", name="h2T")
        h2_sb = big2.tile([P, TBn, E], F16, tag="b2", name="h2_sb")
        ln2_stats = {}
        for tb in range(TBn + 1):
            if tb < TBn:
                ln2_stats[tb] = _emit_ln_stats(
                    nc, ln2p, x2[:, tb], "ln2", f"ln2_{tb}", eps_sb
                )
            if tb >= 1:
                rstd, nmr = ln2_stats[tb - 1]
                nc.scalar.activation(
                    h2_sb[:, tb - 1], x2[:, tb - 1], AF.Identity,
                    bias=nmr[:], scale=rstd[:],
                )
                _emit_tp_block(nc, ps_tp2, tb - 1, h2_sb, h2T, ident16, "h2")
        ln2p.release()
        big2.release()

        if STOP_BEFORE_FFN:
            for c in range(EC):
                w8 = T * mybir.dt.size(h2T.dtype) // 4
                nc.sync.dma_start(
                    out_d[c * P : (c + 1) * P, 0:w8],
                    h2T[:, c, :].bitcast(F32),
                )
            pA_h2.release()
            pA_x2.release()
            constp.release()
            ps_tp2.release()
            ps_attn.release()
            ps_big.release()
            continue

        # ---- Phase 6: FFN + residual + output ----
        # joint token-halves: h1T holds the full hidden state so w1 and w2
        # each stream from HBM exactly once
        h1p = tc.alloc_tile_pool(name="h1p", bufs=1)
        fw1 = tc.alloc_tile_pool(name="fw1", bufs=2)
        h1T = h1p.tile([P, FC, T], F8_2, tag="h1T", name="h1T")
        for k in range(FC):
            if k % 4 == 0:
                w1col = fw1.tile(
                    [P, 4, EC, P], F8, tag="w1col", name=f"w1c_{k // 4}"
                )
                nc.sync.dma_start(
                    w1col[:],
                    w1t_d[k : k + 4].rearrange("kk p i c -> p kk i c"),
                )
            psms = [
                ps_big.tile([P, 512], F32, tag="mm", name=f"h1_ps_{k}_{th}")
                for th in range(2)
            ]
            for i in range(0, EC, 2):
                for th in range(2):
                    nc.tensor.matmul(
                        psms[th][:],
                        w1col[:, k % 4, i : i + 2, :],
                        h2T[:, i : i + 2, th * 512 : (th + 1) * 512],
                        start=(i == 0),
                        stop=(i == EC - 2),
                        perf_mode=DR,
                    )
            for th in range(2):
                nc.scalar.activation(
                    h1T[:, k, th * 512 : (th + 1) * 512],
                    psms[th][:],
                    AF.Relu,
                    bias=b1_sb[:, k : k + 1],
                    scale=ds_1,
                )
        fw1.release()
        pA_h2.release()
        fw2 = tc.alloc_tile_pool(name="fw2", bufs=2)
        fout = tc.alloc_tile_pool(name="fout", bufs=1)
        fftp = tc.alloc_tile_pool(name="fftp", bufs=2)
        ostage = fout.tile([P, TBn, E], F32, tag="ostage", name="ostage")
        for c in range(EC):
            w2col = fw2.tile([P, FC, P], F8_2, tag="w2col", name=f"w2c_{c}")
            nc.sync.dma_start(w2col[:], w2t_d[c])
            psms = [
                ps_big.tile([P, 512], F32, tag="mm", name=f"ff_ps_{c}_{th}")
                for th in range(2)
            ]
            if FFN2_FP8:
                for k in range(0, FC, 2):
                    for th in range(2):
                        nc.tensor.matmul(
                            psms[th][:],
                            w2col[:, k : k + 2, :],
                            h1T[:, k : k + 2, th * 512 : (th + 1) * 512],
                            start=(k == 0),
                            stop=(k == FC - 2),
                            perf_mode=DR,
                        )
            else:
                for k in range(FC):
                    for th in range(2):
                        nc.tensor.matmul(
                            psms[th][:],
                            w2col[:, k],
                            h1T[:, k, th * 512 : (th + 1) * 512],
                            start=(k == 0),
                            stop=(k == FC - 1),
                        )
            for th in range(2):
                ffT = fftp.tile([P, 512], F32, tag="ffT", name=f"ffT_{c}_{th}")
                nc.scalar.activation(
                    ffT[:], psms[th][:], AF.Identity, bias=b2_sb[:, c : c + 1],
                    scale=ds_2,
                )
                for tbl in range(4):
                    tb = th * 4 + tbl
                    tp = ps_tp2.tile([P, P], F32, tag="tp", name=f"f_tp_{c}_{th}_{tbl}")
                    nc.tensor.transpose(
                        tp[:], ffT[:, tbl * P : (tbl + 1) * P], ident[:]
                    )
                    nc.vector.tensor_tensor(
                        ostage[:, tb, c * P : (c + 1) * P],
                        x2[:, tb, c * P : (c + 1) * P],
                        tp[:],
                        op=OP.add,
                    )
            # flush finished output halves early to overlap the store
            if c == 3 or c == EC - 1:
                half = 0 if c == 3 else 1
                for tb in range(TBn):
                    nc.sync.dma_start(
                        out_d[tb * P : (tb + 1) * P, half * 512 : (half + 1) * 512],
                        ostage[:, tb, half * 512 : (half + 1) * 512],
                    )
        fftp.release()
        fout.release()
        fw2.release()
        h1p.release()
        pA_x2.release()
        constp.release()
        ps_tp2.release()
        ps_attn.release()
        ps_big.release()

    nc.compile()
    return nc


_NC = None


def _get_nc():
    global _NC
    if _NC is None:
        _NC = build_nc()
    return _NC


def prepare_in_maps(x, wq, wk, wv, w_proj, b_proj, w1, b1, w2, b2,
                    ln1_g, ln1_b, ln2_g, ln2_b):
    x = np.asarray(x, dtype=np.float32)
    wq2 = np.asarray(wq, dtype=np.float32).reshape(E, E)
    wk2 = np.asarray(wk, dtype=np.float32).reshape(E, E)
    wv2 = np.asarray(wv, dtype=np.float32).reshape(E, E)
    w_proj = np.asarray(w_proj, dtype=np.float32)
    b_proj = np.asarray(b_proj, dtype=np.float32)
    w1 = np.asarray(w1, dtype=np.float32)
    b1 = np.asarray(b1, dtype=np.float32)
    w2 = np.asarray(w2, dtype=np.float32)
    b2 = np.asarray(b2, dtype=np.float32)
    g1 = np.asarray(ln1_g, dtype=np.float32)
    be1 = np.asarray(ln1_b, dtype=np.float32)
    g2 = np.asarray(ln2_g, dtype=np.float32)
    be2 = np.asarray(ln2_b, dtype=np.float32)

    def _tile_w(arr):
        # [K_in, N_out] -> [N_out//P, P(=k_in%P), K_in//P, P] so each DMA reads
        # contiguous per-partition lines
        K, N = arr.shape
        return np.ascontiguousarray(
            arr.reshape(K // P, P, N // P, P).transpose(2, 1, 0, 3)
        )

    scale = np.float32(E) ** -0.5
    # fold LN1 gamma into qkv weights, LN1 beta into qkv biases; fold the
    # attention scale into wq.  V's bias is constant across tokens after
    # softmax (rows sum to 1), so it folds into the proj bias.
    wqt = np.ascontiguousarray((wq2 * g1[None, :] * scale).T)
    wkt = np.ascontiguousarray((wk2 * g1[None, :]).T)
    wvt = np.ascontiguousarray((wv2 * g1[None, :]).T)
    bq = (wq2 @ be1) * scale
    bk = wk2 @ be1
    bv = wv2 @ be1
    wpt = np.ascontiguousarray(w_proj.T)
    bp = b_proj + w_proj @ bv
    w1t = np.ascontiguousarray((w1 * g2[None, :]).T)
    b1e = b1 + w1 @ be2
    w2t = np.ascontiguousarray(w2.T)

    # fp8 e4m3 quantization with exact pow2 pre-scales (descaled on-chip at
    # the PSUM copy-out, so the math is exact apart from the 3-mantissa-bit
    # rounding of each weight)
    for nm, arr in [("wqt", wqt), ("wkt", wkt), ("w1t", w1t), ("w2t", w2t)]:
        _SCALES[nm] = _pow2_scale(arr)

    def _q8(nm, arr):
        return (arr * _SCALES[nm]).astype(NP_F8)

    w2s = (w2t * _SCALES["w2t"]).astype(NP_F8 if FFN2_FP8 else np.float16)

    common = {
        "wqt": _tile_w(_q8("wqt", wqt)),
        "wkt": _tile_w(_q8("wkt", wkt)),
        "wvt": _tile_w(wvt.astype(np.float16)),
        "wpt": _tile_w(wpt.astype(np.float16)),
        "w1t": _tile_w(_q8("w1t", w1t)),
        "w2t": _tile_w(w2s),
        "bq": np.ascontiguousarray(bq.reshape(EC, P).T),
        "bk": np.ascontiguousarray(bk.reshape(EC, P).T),
        "bp": np.ascontiguousarray(bp.reshape(EC, P).T),
        "b1": np.ascontiguousarray(b1e.reshape(FC, P).T),
        "b2": np.ascontiguousarray(b2.reshape(EC, P).T),
    }
    return [dict(common, x=np.ascontiguousarray(x[b])) for b in range(B)]


def kernel(**inputs):
    in_maps = prepare_in_maps(**inputs)
    nc = _get_nc()
    res = run_bass_kernel_spmd(nc, in_maps, core_ids=list(range(B)))
    out = np.stack([res.results[b]["out"] for b in range(B)], axis=0)
    return out.astype(np.float32)



# revision 31
# speedup vs baseline: 1.0593x; 1.0593x over previous
"""Trainium2 Bass kernel for a dense transformer block (LN -> 16-head causal
attention -> proj+residual -> LN -> FFN+residual), B=8 data-parallel over 8
NeuronCores (one batch element per core).

Matmuls run in fp16 (Q/K + FFN in fp8 DoubleRow at 2x PE rate); the residual
stream, layer norms and softmax bookkeeping stay fp32.  LayerNorm gamma/beta
are folded into the adjacent projection weights on the host (exact algebra),
and the 1/sqrt(E) attention scale is folded into wq.

Activation dataflow is transposed ([feature, token]) for the matmul chain;
scores are computed transposed ([key, query]) so softmax denominators come
out of the PV matmul via an appended ones-column in V, removing any need to
transpose the attention probabilities.

Schedule: the LN1 pipeline feeds V matmuls per token block and Q/K matmuls
per token half as transposes land, so the PE is saturated from ~5us on;
attention then runs with all of Q/K/V resident (ACT exp-bound, scores
pipelined 3 deep in PSUM); proj runs token-half-major so LN2 of the first
half overlaps proj of the second; FFN1 runs th-major (w1 streamed twice) so
it chains straight onto LN2.  PSUM copy-outs everywhere are spread over
ACT/DVE/Pool so no single engine gates the PE.
"""

import os
import sys

sys.path.insert(0, "/opt/trn_rl_repo")

# a cpu-pinned jax would hide the NeuronCores from the PJRT execution path
if os.environ.get("JAX_PLATFORMS") == "cpu":
    os.environ.pop("JAX_PLATFORMS")

import numpy as np

import concourse.bacc as bacc
import concourse.mybir as mybir
import concourse.tile as tile
from concourse.bass_utils import run_bass_kernel_spmd
from concourse.masks import make_identity

F32 = mybir.dt.float32
F16 = mybir.dt.float16
F8 = mybir.dt.float8e4
DR = mybir.MatmulPerfMode.DoubleRow
AF = mybir.ActivationFunctionType
OP = mybir.AluOpType
AX = mybir.AxisListType

NP_F8 = mybir.dt.np(F8)  # ml_dtypes.float8_e4m3 (IEEE, max 240, min normal 2^-6)

# fp8 weight pre-scales (power of 2, exact): w8 = w * S, descale 1/S applied at
# the PSUM copy-out.  Set by prepare_in_maps before build_nc is called.
_SCALES: dict = {}


def _pow2_scale(arr, target=8.0):
    m = float(np.abs(arr).max())
    if m == 0.0:
        return 1.0
    return 2.0 ** np.floor(np.log2(target / m))

# phase marker for the sim analyzer (no effect on the kernel itself)
_PHASE = ["init"]

P = 128
E = 1024
H = 16
D = 64
T = 1024
B = 8
F = 4 * E
EC = E // P     # 8 chunks of features
FC = F // P     # 32 chunks of ffn hidden
TBn = T // P    # 8 token blocks of 128
LN_EPS = 1e-5
NEG = -1.0e9


def _emit_ln_stats(nc, pool, src_blk, tag, name, eps_sb):
    """Per-token-block LN stats on DVE only (bn_stats fused mean/var):
    src_blk [128, E] -> (rstd, nmr) [128, 1]."""
    st6 = pool.tile([P, 2, 6], F32, tag=f"{tag}_st6", name=f"{name}_st6")
    mv = pool.tile([P, 2], F32, tag=f"{tag}_mv", name=f"{name}_mv")
    rstd = pool.tile([P, 1], F32, tag=f"{tag}_rstd", name=f"{name}_rstd")
    nmr = pool.tile([P, 1], F32, tag=f"{tag}_nmr", name=f"{name}_nmr")
    src2 = src_blk.rearrange("p (g c) -> p g c", g=2)
    nc.vector.bn_stats(st6[:, 0], src2[:, 0])
    nc.vector.bn_stats(st6[:, 1], src2[:, 1])
    nc.vector.bn_aggr(mv[:], st6[:])
    nc.scalar.activation(rstd[:], mv[:, 1:2], AF.Sqrt, bias=eps_sb[:])
    nc.vector.reciprocal(rstd[:], rstd[:])
    nc.vector.scalar_tensor_tensor(
        nmr[:], mv[:, 0:1], -1.0, rstd[:], op0=OP.mult, op1=OP.mult
    )
    return rstd, nmr


def _emit_tp_block(nc, ps_tp, tb, src, dst, ident, name, dst2=None):
    """One token block of src [128, TBn, E] (token-major) -> dst[, dst2]
    [128, EC, T] (feature-major) via PE transposes of 128x128 blocks; the
    copies out of PSUM alternate engines (and cast to the dst dtype)."""
    for j in range(EC):
        tp = ps_tp.tile([P, P], src.dtype, tag="tp", name=f"{name}_tp_{j}_{tb}")
        nc.tensor.transpose(tp[:], src[:, tb, j * P : (j + 1) * P], ident[:])
        if (j + tb) % 2 == 0:
            nc.vector.tensor_copy(dst[:, j, tb * P : (tb + 1) * P], tp[:])
            if dst2 is not None:
                nc.scalar.copy(dst2[:, j, tb * P : (tb + 1) * P], tp[:])
        else:
            nc.scalar.copy(dst[:, j, tb * P : (tb + 1) * P], tp[:])
            if dst2 is not None:
                nc.vector.tensor_copy(dst2[:, j, tb * P : (tb + 1) * P], tp[:])


def build_nc(reps=1):
    nc = bacc.Bacc(None, target_bir_lowering=False)

    assert _SCALES, "prepare_in_maps must run before build_nc (sets fp8 scales)"
    ds_q = 1.0 / _SCALES["wqt"]
    ds_k = 1.0 / _SCALES["wkt"]
    # h1T stores S1*h1 (S1 = w1's fp8 pre-scale, fixed 64): the FFN1 copy-out
    # is then max(psum + S1*b1, 0) -- a 2-op tensor_scalar any engine can run.
    # The 1/S1 descale folds into FFN2's copy-out scale.
    ds_2 = 1.0 / (_SCALES["w1t"] * _SCALES["w2t"])

    x_d = nc.dram_tensor("x", [T, E], F32, kind="ExternalInput")
    # weights pre-tiled on host to [out_chunk, p(=in%128), in_chunk, col]
    wqt_d = nc.dram_tensor("wqt", [EC, P, EC, P], F8, kind="ExternalInput")
    wkt_d = nc.dram_tensor("wkt", [EC, P, EC, P], F8, kind="ExternalInput")
    # wvt comes pre-tiled in moving layout [p(=in%128), in_chunk, out_chunk, col]
    wvt_d = nc.dram_tensor("wvt", [P, EC, EC, P], F16, kind="ExternalInput")
    wpt_d = nc.dram_tensor("wpt", [EC, P, EC, P], F16, kind="ExternalInput")
    w1t_d = nc.dram_tensor("w1t", [FC, P, EC, P], F8, kind="ExternalInput")
    w2t_d = nc.dram_tensor("w2t", [EC, P, FC, P], F8, kind="ExternalInput")
    bq_d = nc.dram_tensor("bq", [P, EC], F32, kind="ExternalInput")
    bk_d = nc.dram_tensor("bk", [P, EC], F32, kind="ExternalInput")
    bp_d = nc.dram_tensor("bp", [P, EC], F32, kind="ExternalInput")
    b1_d = nc.dram_tensor("b1", [P, FC], F32, kind="ExternalInput")
    b2_d = nc.dram_tensor("b2", [P, EC], F32, kind="ExternalInput")
    out_d = nc.dram_tensor("out", [T, E], F32, kind="ExternalOutput")

    with tile.TileContext(nc) as tc:
      for _rep in range(reps):
        _PHASE[0] = "init"
        # -------- pools (LIFO per side) --------
        ps_qk = tc.alloc_tile_pool(name="ps_qk", bufs=2, space="PSUM")
        ps_tp = tc.alloc_tile_pool(name="ps_tp", bufs=2, space="PSUM")
        ps_v = tc.alloc_tile_pool(name="ps_v", bufs=2, space="PSUM")
        constp = tc.alloc_tile_pool(name="const", bufs=1)
        wpp = tc.alloc_tile_pool(name="wpp", bufs=1)
        big2 = tc.alloc_tile_pool(name="big2", bufs=3)
        qkp = tc.alloc_tile_pool(name="qkp", bufs=1)
        big1 = tc.alloc_tile_pool(name="big1", bufs=1)
        wqk_p = tc.alloc_tile_pool(name="wqk_p", bufs=1)
        wv_p = tc.alloc_tile_pool(name="wv_p", bufs=1)
        ln1p = tc.alloc_tile_pool(name="ln1", bufs=2)

        # -------- input / weight DMA, interleaved so LN1 and the V matmuls
        # are fed in arrival order --------
        x_sb = big2.tile([P, TBn, E], F32, tag="b2", name="x_sb")
        wv_sb = wv_p.tile([P, EC, EC, P], F16, tag="wv", name="wv_sb")
        wq_sb = wqk_p.tile([P, EC, EC, P], F8, tag="wq", name="wq_sb")
        wk_sb = wqk_p.tile([P, EC, EC, P], F8, tag="wk", name="wk_sb")

        nc.sync.dma_start(x_sb[:, 0], x_d[0:P, :])
        nc.sync.dma_start(x_sb[:, 1], x_d[P : 2 * P, :])
        # wv laid out moving-side: [p(=in%128), i, jj, c]
        nc.sync.dma_start(wv_sb[:, :, 0:4], wvt_d[:, :, 0:4])
        nc.sync.dma_start(x_sb[:, 2], x_d[2 * P : 3 * P, :])
        nc.sync.dma_start(wv_sb[:, :, 4:8], wvt_d[:, :, 4:8])
        nc.sync.dma_start(x_sb[:, 3], x_d[3 * P : 4 * P, :])
        nc.sync.dma_start(wq_sb[:], wqt_d.rearrange("j p i c -> p j i c"))
        nc.sync.dma_start(x_sb[:, 4], x_d[4 * P : 5 * P, :])
        nc.sync.dma_start(wk_sb[:], wkt_d.rearrange("j p i c -> p j i c"))
        for tb in range(5, TBn):
            nc.sync.dma_start(x_sb[:, tb], x_d[tb * P : (tb + 1) * P, :])

        # -------- constants --------
        ident16 = constp.tile([P, P], F16, name="ident16")
        make_identity(nc, ident16[:])
        eps_sb = constp.tile([P, 1], F32, name="eps_sb")
        nc.vector.memset(eps_sb[:], LN_EPS)
        # mask[s, q] = 0 if s <= q else NEG  (within a diagonal 128 block)
        mask = constp.tile([P, P], F32, name="mask")
        nc.gpsimd.memset(mask[:], 0.0)
        nc.gpsimd.affine_select(
            out=mask[:],
            in_=mask[:],
            compare_op=OP.is_ge,
            fill=NEG,
            base=0,
            pattern=[[1, P]],
            channel_multiplier=-1,
        )
        bq_sb = constp.tile([P, EC], F32, name="bq_sb")
        bk_sb = constp.tile([P, EC], F32, name="bk_sb")
        bp_sb = constp.tile([P, EC], F32, name="bp_sb")
        b1_sb = constp.tile([P, FC], F32, name="b1_sb")
        b2_sb = constp.tile([P, EC], F32, name="b2_sb")
        nc.sync.dma_start(bq_sb[:], bq_d[:])
        nc.sync.dma_start(bk_sb[:], bk_d[:])
        # proj weights + late biases are DMA'd at attention start so they
        # don't delay the x / qkv-weight stream phase 1 depends on
        wp_sb = wpp.tile([P, EC, EC, P], F16, tag="wp", name="wp_sb")

        # -------- phase 1: LN1 feeding V (per token block) and Q/K (per
        # token half) as the transposed activations land --------
        _PHASE[0] = "ln1"
        h_sb = big2.tile([P, TBn, E], F16, tag="b2", name="h_sb")
        V_pad = big2.tile([P, TBn, H * (D + 1)], F16, tag="b2", name="V_pad")
        vpr = V_pad[:].rearrange("p tb (h dd) -> p tb h dd", dd=D + 1)
        ones16 = constp.tile([P, TBn, H], F32, name="ones16")
        nc.vector.memset(ones16[:], 1.0)
        nc.vector.tensor_copy(vpr[:, :, :, D : D + 1], ones16[:, :, :, None])
        hT = big1.tile([P, EC, T], F16, tag="b1", name="hT")
        hT8 = big1.tile([P, EC, T], F8, tag="b1_8", name="hT8")
        qt = [
            qkp.tile([P, T], F16, tag=f"qt{j}", name=f"QT_{_rep}_{j}")
            for j in range(EC)
        ]
        kt = [
            qkp.tile([P, T], F16, tag=f"kt{j}", name=f"KT_{_rep}_{j}")
            for j in range(EC)
        ]

        qk_rot = [0]

        def emit_qk_half(tq):
            """Q/K projections for one token half: all 8 feature chunks of
            each, fp8 DoubleRow, copy-outs rotating over ACT/DVE/Pool."""
            sl = slice(tq * 512, (tq + 1) * 512)
            for j in range(EC):
                for w_sb, dst, bias_sb, dsc, wn in (
                    (wk_sb, kt, bk_sb, ds_k, "k"),
                    (wq_sb, qt, bq_sb, ds_q, "q"),
                ):
                    psm = ps_qk.tile(
                        [P, 512], F32, tag="mm",
                        name=f"qk_ps_{_rep}_{wn}_{j}_{tq}",
                    )
                    for i in range(0, EC, 2):
                        nc.tensor.matmul(
                            psm[:],
                            w_sb[:, j, i : i + 2, :],
                            hT8[:, i : i + 2, sl],
                            start=(i == 0),
                            stop=(i == EC - 2),
                            perf_mode=DR,
                        )
                    sel = qk_rot[0] % 2
                    qk_rot[0] += 1
                    if sel == 0:
                        nc.vector.tensor_scalar(
                            dst[j][:, sl], psm[:], dsc, bias_sb[:, j : j + 1],
                            op0=OP.mult, op1=OP.add,
                        )
                    else:
                        nc.scalar.activation(
                            dst[j][:, sl], psm[:], AF.Identity,
                            bias=bias_sb[:, j : j + 1], scale=dsc,
                        )

        ln1_stats = {}
        for tb in range(TBn + 1):
            if tb < TBn:
                ln1_stats[tb] = _emit_ln_stats(
                    nc, ln1p, x_sb[:, tb], "ln1", f"ln1_{tb}", eps_sb
                )
            if tb >= 1:
                m = tb - 1
                rstd, nmr = ln1_stats[m]
                nc.scalar.activation(
                    h_sb[:, m], x_sb[:, m], AF.Identity,
                    bias=nmr[:], scale=rstd[:],
                )
                _emit_tp_block(nc, ps_tp, m, h_sb, hT, ident16, "h")
                # fp8 shadow of hT for the Q/K DoubleRow matmuls, produced on
                # the otherwise-idle gpsimd engine (SBUF->SBUF)
                nc.gpsimd.tensor_copy(
                    hT8[:, :, m * P : (m + 1) * P],
                    hT[:, :, m * P : (m + 1) * P],
                )
                # V for all 16 heads of this token block: one [P,1024] psum,
                # two bank-halves, contraction over all 8 feature chunks
                psv = ps_v.tile([P, 1024], F32, tag="v", name=f"v_ps_{_rep}_{m}")
                for i in range(EC):
                    for vh in range(2):
                        nc.tensor.matmul(
                            psv[:, vh * 512 : (vh + 1) * 512],
                            hT[:, i, m * P : (m + 1) * P],
                            wv_sb[:, i, 4 * vh : 4 * vh + 4, :],
                            start=(i == 0),
                            stop=(i == EC - 1),
                        )
                dstv = vpr[:, m, :, 0:D]
                srcv = psv[:].rearrange("p (h d) -> p h d", d=D)
                if m % 2 == 0:
                    nc.vector.tensor_copy(dstv, srcv)
                else:
                    nc.scalar.copy(dstv, srcv)
                if m == 3:
                    emit_qk_half(0)
                if m == TBn - 1:
                    emit_qk_half(1)
        ln1p.release()
        wv_p.release()
        wqk_p.release()
        big1.release()
        ps_v.release()
        ps_tp.release()
        ps_qk.release()

        # -------- phase 2: attention, ACT exp-bound with 3-deep score
        # pipelining; causal mask pre-written into PSUM so the diagonal
        # score matmuls accumulate straight onto it --------
        _PHASE[0] = "attn"
        nc.sync.dma_start(wp_sb[:], wpt_d.rearrange("c p i cc -> p c i cc"))
        nc.sync.dma_start(bp_sb[:], bp_d[:])
        nc.sync.dma_start(b1_sb[:], b1_d[:])
        nc.sync.dma_start(b2_sb[:], b2_d[:])
        expp = tc.alloc_tile_pool(name="expp", bufs=6)
        rp = tc.alloc_tile_pool(name="rp", bufs=2)
        ps_sc = tc.alloc_tile_pool(name="ps_sc", bufs=3, space="PSUM")
        ps_attn = tc.alloc_tile_pool(name="ps_attn", bufs=2, space="PSUM")

        attnT = big2.tile([P, EC, T], F16, tag="b2", name="attnT")
        for j in range(EC):
            QTj, KTj = qt[j], kt[j]
            for qc in range(2):
                nblk = 4 * qc + 4
                aps = [
                    ps_attn.tile(
                        [D + 1, 512], F32, tag="attn",
                        name=f"at_ps_{_rep}_{j}_{qc}_{hh}",
                    )
                    for hh in range(2)
                ]
                exps = {}
                offs = {}
                for sb in range(nblk):
                    off = max(0, (sb - 4 * qc) * P)
                    offs[sb] = off
                    # both heads share one 2-bank psum tile: h0 cols
                    # [off,512), h1 cols [512+off,1024)
                    sc = ps_sc.tile(
                        [P, 1024], F32, tag="sc",
                        name=f"sc_{_rep}_{j}_{qc}_{sb}",
                    )
                    scv = sc[:].rearrange("p (g c) -> p g c", g=2)
                    diag = sb >= 4 * qc
                    for hh in range(2):
                        nc.tensor.matmul(
                            sc[:, hh * 512 + off : (hh + 1) * 512],
                            KTj[hh * D : (hh + 1) * D, sb * P : (sb + 1) * P],
                            QTj[hh * D : (hh + 1) * D,
                                qc * 512 + off : (qc + 1) * 512],
                            start=True,
                            stop=True,
                        )
                    if diag:
                        nc.vector.tensor_tensor(
                            scv[:, :, off : off + P],
                            scv[:, :, off : off + P],
                            mask[:, None, :].to_broadcast([P, 2, P]),
                            op=OP.add,
                        )
                    ex = expp.tile(
                        [P, 1024], F16, tag="exp",
                        name=f"ex_{_rep}_{j}_{qc}_{sb}",
                    )
                    exv = ex[:].rearrange("p (g c) -> p g c", g=2)
                    nc.scalar.activation(
                        exv[:, :, off:512], scv[:, :, off:512], AF.Exp
                    )
                    exps[sb] = ex
                    # software-pipeline: PV of sb-1 issued after scores of sb
                    if sb > 0:
                        for hh in range(2):
                            po = offs[sb - 1]
                            nc.tensor.matmul(
                                aps[hh][:, po:512],
                                V_pad[:, sb - 1,
                                      (2 * j + hh) * (D + 1)
                                      : (2 * j + hh + 1) * (D + 1)],
                                exps[sb - 1][:, hh * 512 + po
                                             : (hh + 1) * 512],
                                start=(sb - 1 == 0),
                                stop=False,
                            )
                for hh in range(2):
                    po = offs[nblk - 1]
                    nc.tensor.matmul(
                        aps[hh][:, po:512],
                        V_pad[:, nblk - 1,
                              (2 * j + hh) * (D + 1)
                              : (2 * j + hh + 1) * (D + 1)],
                        exps[nblk - 1][:, hh * 512 + po : (hh + 1) * 512],
                        start=(nblk == 1),
                        stop=True,
                    )
                for hh in range(2):
                    # drain psum to SBUF right away so the aps slot frees for
                    # the next query group; normalize runs from the SBUF copy
                    asb = rp.tile(
                        [D + 1, 512], F32, tag="asb", bufs=4,
                        name=f"as_{_rep}_{j}_{qc}_{hh}",
                    )
                    if hh == 0:
                        nc.vector.tensor_copy(asb[:], aps[hh][:])
                    else:
                        nc.scalar.copy(asb[:], aps[hh][:])
                    rinv = rp.tile(
                        [1, 512], F32, tag="rinv", name=f"ri_{_rep}_{j}_{qc}_{hh}"
                    )
                    nc.vector.reciprocal(rinv[:], asb[D : D + 1, :])
                    rb = rp.tile(
                        [D, 512], F32, tag="rb", name=f"rb_{_rep}_{j}_{qc}_{hh}"
                    )
                    nc.gpsimd.partition_broadcast(rb[:], rinv[:])
                    eng = nc.vector if (2 * qc + hh) % 2 == 0 else nc.gpsimd
                    eng.tensor_tensor(
                        attnT[hh * D : (hh + 1) * D, j,
                              qc * 512 : (qc + 1) * 512],
                        asb[0:D, :],
                        rb[:],
                        op=OP.mult,
                    )
        rp.release()
        expp.release()
        qkp.release()
        ps_attn.release()
        ps_sc.release()

        ps_big = tc.alloc_tile_pool(name="ps_big", bufs=2, space="PSUM")
        ps_tp2 = tc.alloc_tile_pool(name="ps_tp2", bufs=2, space="PSUM")
        ps_ffn = tc.alloc_tile_pool(name="ps_ffn", bufs=4, space="PSUM")

        # -------- phase 3+4: proj + LN2 + FFN, token-half pipelined:
        #   proj(half0) -> LN2(half0) -> proj(half1) -> FFN1(half0)
        #   -> LN2(half1) -> FFN1(half1) -> FFN2 --------
        _PHASE[0] = "proj"
        pA_x2 = tc.alloc_tile_pool(name="pA_x2", bufs=1, side="right")
        h1p = tc.alloc_tile_pool(name="h1p", bufs=1, side="right")
        pA_h2 = tc.alloc_tile_pool(name="pA_h2", bufs=1, side="right")
        fw1 = tc.alloc_tile_pool(name="fw1", bufs=3, side="right")
        satp = tc.alloc_tile_pool(name="satp", bufs=3)
        ln2p = tc.alloc_tile_pool(name="ln2", bufs=2)
        x2 = pA_x2.tile([P, TBn, E], F32, tag="bA_x2", name="x2")
        h1T = h1p.tile([P, FC, T], F8, tag="h1T", name="h1T")
        h2T = pA_h2.tile([P, EC, T], F8, tag="bA_h2T", name="h2T")
        h2_sb = big2.tile([P, TBn, E], F16, tag="b2", name="h2_sb")

        # w1 chunk prefetch, 2 ahead of consumption
        w1seq = [(th, ch) for th in range(2) for ch in range(FC // 4)]
        w1tiles = {}

        def w1_prefetch(n):
            if n < len(w1seq) and w1seq[n] not in w1tiles:
                th, ch = w1seq[n]
                t = fw1.tile(
                    [P, 4, EC, P], F8, tag="w1col", name=f"w1c_{th}_{ch}"
                )
                nc.sync.dma_start(
                    t[:],
                    w1t_d[4 * ch : 4 * ch + 4].rearrange("kk p i c -> p kk i c"),
                )
                w1tiles[(th, ch)] = t

        def emit_proj_half(tq):
            """proj for one token half, transposes pipelined one column
            behind the matmuls so the PE never waits on the ACT copy-out."""
            pend = []

            def flush_pend():
                for c, saT in pend:
                    tp = ps_tp2.tile([P, 512], F16, tag="tp", name=f"pj_tp_{c}")
                    for tbl in range(4):
                        nc.tensor.transpose(
                            tp[:, tbl * P : (tbl + 1) * P],
                            saT[:, tbl * P : (tbl + 1) * P],
                            ident16[:],
                        )
                    tb0 = tq * 4
                    nc.vector.tensor_tensor(
                        x2[:, tb0 : tb0 + 4, c * P : (c + 1) * P],
                        x_sb[:, tb0 : tb0 + 4, c * P : (c + 1) * P],
                        tp[:].rearrange("p (b c) -> p b c", b=4),
                        op=OP.add,
                    )
                pend.clear()

            for c in range(EC):
                saT = satp.tile([P, 512], F16, tag="saT", name=f"saT_{c}_{tq}")
                psm = ps_big.tile([P, 512], F32, tag="mm", name=f"pj_ps_{c}_{tq}")
                for i in range(EC):
                    nc.tensor.matmul(
                        psm[:],
                        wp_sb[:, c, i],
                        attnT[:, i, tq * 512 : (tq + 1) * 512],
                        start=(i == 0),
                        stop=(i == EC - 1),
                    )
                nc.scalar.activation(
                    saT[:], psm[:], AF.Identity, bias=bp_sb[:, c : c + 1]
                )
                flush_pend()
                pend.append((c, saT))
            flush_pend()

        def emit_ln2_half(tq):
            _PHASE[0] = "ln2"
            stats = {}
            for tbl in range(4):
                tb = tq * 4 + tbl
                stats[tb] = _emit_ln_stats(
                    nc, ln2p, x2[:, tb], "ln2", f"ln2_{tb}", eps_sb
                )
            for tbl in range(4):
                tb = tq * 4 + tbl
                rstd, nmr = stats[tb]
                nc.scalar.activation(
                    h2_sb[:, tb], x2[:, tb], AF.Identity,
                    bias=nmr[:], scale=rstd[:],
                )
                _emit_tp_block(nc, ps_tp2, tb, h2_sb, h2T, ident16, "h2")

        def emit_ffn1_half(th):
            _PHASE[0] = "ffn1"
            for k in range(FC):
                if k % 4 == 0:
                    w1_prefetch(th * (FC // 4) + k // 4 + 2)
                w1col = w1tiles[(th, k // 4)]
                psm = ps_ffn.tile(
                    [P, 512], F32, tag="mm", name=f"h1_ps_{k}_{th}"
                )
                for i in range(0, EC, 2):
                    nc.tensor.matmul(
                        psm[:],
                        w1col[:, k % 4, i : i + 2, :],
                        h2T[:, i : i + 2, th * 512 : (th + 1) * 512],
                        start=(i == 0),
                        stop=(i == EC - 2),
                        perf_mode=DR,
                    )
                # h1T = max(psum + S1*b1, 0) = S1*relu(w1 h2 + b1); copy-outs
                # rotate over ACT/DVE/Pool so no engine gates the PE
                dst = h1T[:, k, th * 512 : (th + 1) * 512]
                if (2 * k + th) % 2 == 0:
                    nc.scalar.activation(
                        dst, psm[:], AF.Relu, bias=b1_sb[:, k : k + 1]
                    )
                else:
                    nc.vector.tensor_scalar(
                        dst, psm[:], b1_sb[:, k : k + 1], 0.0,
                        op0=OP.add, op1=OP.max,
                    )

        emit_proj_half(0)
        emit_ln2_half(0)
        _PHASE[0] = "proj"
        w1_prefetch(0)
        w1_prefetch(1)
        emit_proj_half(1)
        emit_ffn1_half(0)
        emit_ln2_half(1)
        ln2p.release()
        satp.release()
        big2.release()
        wpp.release()

        # -------- FFN2 per token half, pipelined against FFN1 of the other
        # half; w2 streamed once per half --------
        fw2 = tc.alloc_tile_pool(name="fw2", bufs=3)
        fout = tc.alloc_tile_pool(name="fout", bufs=1)
        fftp = tc.alloc_tile_pool(name="fftp", bufs=2)
        ostage = fout.tile([P, TBn, E], F32, tag="ostage", name="ostage")

        w2tiles = {}

        def w2_prefetch(th, c):
            if c < EC and (th, c) not in w2tiles:
                t = fw2.tile([P, FC, P], F8, tag="w2col", name=f"w2c_{th}_{c}")
                nc.sync.dma_start(t[:], w2t_d[c])
                w2tiles[(th, c)] = t

        def emit_ffn2_half(th):
            _PHASE[0] = "ffn2"
            pend2 = []

            def flush_pend2():
                for c, ffT in pend2:
                    tp = ps_tp2.tile(
                        [P, 512], F16, tag="tp", name=f"f_tp_{c}_{th}"
                    )
                    for tbl in range(4):
                        nc.tensor.transpose(
                            tp[:, tbl * P : (tbl + 1) * P],
                            ffT[:, tbl * P : (tbl + 1) * P],
                            ident16[:],
                        )
                    tb0 = th * 4
                    nc.vector.tensor_tensor(
                        ostage[:, tb0 : tb0 + 4, c * P : (c + 1) * P],
                        x2[:, tb0 : tb0 + 4, c * P : (c + 1) * P],
                        tp[:].rearrange("p (b c) -> p b c", b=4),
                        op=OP.add,
                    )
                    if th == 1:
                        # store completed columns: pairs mid-stream (fewer
                        # DMA setups), singles for the last two (short tail)
                        if c >= EC - 2:
                            for tb in range(TBn):
                                nc.sync.dma_start(
                                    out_d[tb * P : (tb + 1) * P,
                                          c * P : (c + 1) * P],
                                    ostage[:, tb, c * P : (c + 1) * P],
                                )
                        elif c % 2 == 1:
                            for tb in range(TBn):
                                nc.sync.dma_start(
                                    out_d[tb * P : (tb + 1) * P,
                                          (c - 1) * P : (c + 1) * P],
                                    ostage[:, tb, (c - 1) * P : (c + 1) * P],
                                )
                pend2.clear()

            w2_prefetch(th, 0)
            w2_prefetch(th, 1)
            for c in range(EC):
                w2_prefetch(th, c + 2)
                w2col = w2tiles[(th, c)]
                psm = ps_ffn.tile(
                    [P, 512], F32, tag="mm", name=f"ff_ps_{c}_{th}"
                )
                for k in range(0, FC, 2):
                    nc.tensor.matmul(
                        psm[:],
                        w2col[:, k : k + 2, :],
                        h1T[:, k : k + 2, th * 512 : (th + 1) * 512],
                        start=(k == 0),
                        stop=(k == FC - 2),
                        perf_mode=DR,
                    )
                ffT = fftp.tile([P, 512], F16, tag="ffT", name=f"ffT_{c}_{th}")
                nc.scalar.activation(
                    ffT[:], psm[:], AF.Identity, bias=b2_sb[:, c : c + 1],
                    scale=ds_2,
                )
                flush_pend2()
                pend2.append((c, ffT))
            flush_pend2()

        emit_ffn2_half(0)
        emit_ffn1_half(1)
        fw1.release()
        pA_h2.release()
        emit_ffn2_half(1)
        fftp.release()
        fout.release()
        fw2.release()
        constp.release()
        h1p.release()
        pA_x2.release()
        ps_ffn.release()
        ps_tp2.release()
        ps_big.release()

    nc.compile()
    return nc


_NC = None


def _get_nc():
    global _NC
    if _NC is None:
        _NC = build_nc()
    return _NC


def prepare_in_maps(x, wq, wk, wv, w_proj, b_proj, w1, b1, w2, b2,
                    ln1_g, ln1_b, ln2_g, ln2_b):
    x = np.asarray(x, dtype=np.float32)
    wq2 = np.asarray(wq, dtype=np.float32).reshape(E, E)
    wk2 = np.asarray(wk, dtype=np.float32).reshape(E, E)
    wv2 = np.asarray(wv, dtype=np.float32).reshape(E, E)
    w_proj = np.asarray(w_proj, dtype=np.float32)
    b_proj = np.asarray(b_proj, dtype=np.float32)
    w1 = np.asarray(w1, dtype=np.float32)
    b1 = np.asarray(b1, dtype=np.float32)
    w2 = np.asarray(w2, dtype=np.float32)
    b2 = np.asarray(b2, dtype=np.float32)
    g1 = np.asarray(ln1_g, dtype=np.float32)
    be1 = np.asarray(ln1_b, dtype=np.float32)
    g2 = np.asarray(ln2_g, dtype=np.float32)
    be2 = np.asarray(ln2_b, dtype=np.float32)

    def _tile_w(arr):
        # [K_in, N_out] -> [N_out//P, P(=k_in%P), K_in//P, P] so each DMA reads
        # contiguous per-partition lines
        K, N = arr.shape
        return np.ascontiguousarray(
            arr.reshape(K // P, P, N // P, P).transpose(2, 1, 0, 3)
        )

    scale = np.float32(E) ** -0.5
    # fold LN1 gamma into qkv weights, LN1 beta into qkv biases; fold the
    # attention scale into wq.  V's bias is constant across tokens after
    # softmax (rows sum to 1), so it folds into the proj bias.
    wqt = np.ascontiguousarray((wq2 * g1[None, :] * scale).T)
    wkt = np.ascontiguousarray((wk2 * g1[None, :]).T)
    wvt = np.ascontiguousarray((wv2 * g1[None, :]).T)
    bq = (wq2 @ be1) * scale
    bk = wk2 @ be1
    bv = wv2 @ be1
    wpt = np.ascontiguousarray(w_proj.T)
    bp = b_proj + w_proj @ bv
    w1t = np.ascontiguousarray((w1 * g2[None, :]).T)
    b1e = b1 + w1 @ be2
    w2t = np.ascontiguousarray(w2.T)

    # fp8 e4m3 quantization with exact pow2 pre-scales (descaled on-chip at
    # the PSUM copy-out, so the math is exact apart from the 3-mantissa-bit
    # rounding of each weight)
    for nm, arr in [("wqt", wqt), ("wkt", wkt), ("w2t", w2t)]:
        _SCALES[nm] = _pow2_scale(arr)
    # fixed pow2 scale for w1 so that S1*h1 stays well inside fp8e4 range
    # (h1 max ~1.5 -> 64*h1 <= ~100 < 240) while w1*64 (max ~1) keeps full
    # fp8 relative precision
    _SCALES["w1t"] = 64.0

    def _q8(nm, arr):
        return (arr * _SCALES[nm]).astype(NP_F8)

    common = {
        "wqt": _tile_w(_q8("wqt", wqt)),
        "wkt": _tile_w(_q8("wkt", wkt)),
        # moving layout for the all-heads V matmul: [p, in_chunk, out_chunk, c]
        "wvt": np.ascontiguousarray(
            wvt.astype(np.float16)
            .reshape(EC, P, EC, P)
            .transpose(1, 0, 2, 3)
        ),
        "wpt": _tile_w(wpt.astype(np.float16)),
        "w1t": _tile_w(_q8("w1t", w1t)),
        "w2t": _tile_w(_q8("w2t", w2t)),
        "bq": np.ascontiguousarray(bq.reshape(EC, P).T),
        "bk": np.ascontiguousarray(bk.reshape(EC, P).T),
        "bp": np.ascontiguousarray(bp.reshape(EC, P).T),
        # pre-scaled by S1 so the FFN1 copy-out is max(psum + S1*b1, 0)
        "b1": np.ascontiguousarray((_SCALES["w1t"] * b1e).reshape(FC, P).T),
        "b2": np.ascontiguousarray(b2.reshape(EC, P).T),
    }
    return [dict(common, x=np.ascontiguousarray(x[b])) for b in range(B)]


def kernel(**inputs):
    in_maps = prepare_in_maps(**inputs)
    nc = _get_nc()
    res = run_bass_kernel_spmd(nc, in_maps, core_ids=list(range(B)))
    out = np.stack([res.results[b]["out"] for b in range(B)], axis=0)
    return out.astype(np.float32)
